# revision 30
# baseline (speedup 1.0000x reference)
"""Trainium2 Bass kernel: 3x3 stride-1 pad-1 Conv2d, 16->16 channels, 1024x1024.

Strategy (8 NeuronCores, spatial split over H):
  - Core i computes output rows [128*i, 128*i+128). Its input slice holds rows
    128*i-1 .. 128*i+132 (1-row halo + zero padding), pre-padded on the host
    with one zero column on each side so horizontal taps are free-dim shifts.
  - Windows of 8 input rows advance 7 output rows: one SBUF tile of
    [128 part = (row p 0..7) x (ch c 0..15), 1032 free] feeds 3 accumulating
    matmuls (one per horizontal tap kw) per 512-wide half against a banded
    [128,128] weight matrix. lhsT columns 0..95 hold 6 full output rows;
    96..111 the window's own split row (taps kh=0,1); 112..127 the PREVIOUS
    window's split row kh=2 tap. 7 rows/window vs 6 cuts matmuls 132 -> 114.
  - kw-major matmul order shares each LDWEIGHTS across both 512-halves,
    dropping matmul spacing to the ~215 ns streaming floor.
  - Whole 128-partition PSUM tiles are copied (fp32->fp16) and DMA'd to a
    per-window DRAM buffer; the HOST adds the 18 split-row partial pairs
    during reassembly (engines cannot address PSUM/SBUF partition base 112,
    so an on-device add would need an extra DMA realign + vector add chain
    that measurably outweighs the matmul savings).
  - Engines: SP = input DMA, ACT = h0 copy + odd out-DMA, DVE = h1 copy,
    Pool(gpsimd) = even out-DMA, PE = matmul. fp16 output halves write
    traffic (~3e-4 extra rel err); host upcasts to fp32.
"""

import sys

sys.path.insert(0, "/opt/trn_rl_repo")

import numpy as np

import concourse.bass as bass  # noqa: F401  (engine handles live on nc)
import concourse.mybir as mybir
import concourse.tile as tile
from concourse import bacc
from concourse.bass_utils import run_bass_kernel_spmd

C = 16          # channels in/out
H = 1024        # image height/width
W = 1024
NCORES = 8
RPC = H // NCORES       # output rows per core = 128
SROWS = 134             # input slice rows per core (halo + zero pad)
WPAD = 1032             # padded row width (col 0 and 1025 are zeros, 1..1024 data)

_CACHE = {}


def _build_nc(out_dt: str = "float16", order: str = "kw", adv: int = 7,
              bufs=(8, 4, 6), in_dt: str = "float16"):
    key = ("nc", out_dt, order, adv, bufs, in_dt)
    if key in _CACHE:
        return _CACHE[key]
    nc = bacc.Bacc("TRN2", target_bir_lowering=False, debug=False)
    f32 = mybir.dt.float32
    fin = getattr(mybir.dt, in_dt)
    fout = getattr(mybir.dt, out_dt)
    groups = 19 if adv == 7 else 22
    xs = nc.dram_tensor("xs", [SROWS, C, WPAD], fin, kind="ExternalInput").ap()
    wpk = nc.dram_tensor("wpk", [128, 3 * 128], fin, kind="ExternalInput").ap()
    oshape = [groups * 128, W] if adv == 7 else [RPC * C, W]
    out = nc.dram_tensor("out", oshape, fout, kind="ExternalOutput").ap()

    with tile.TileContext(nc) as tc:
        with (
            tc.tile_pool(name="wp", bufs=1) as wp,
            tc.tile_pool(name="xin", bufs=bufs[0]) as xin,
            tc.tile_pool(name="ps", bufs=bufs[1], space="PSUM") as ps,
            tc.tile_pool(name="stg", bufs=bufs[2]) as stgp,
        ):
            xsf = xs.flatten_outer_dims()  # [SROWS*C, WPAD]
            of = out

            # first input tile before the weight DMA (first matmul's critical
            # path is the big xt0 transfer), split in two so the first half-
            # window can start early; weights go on the ACT ring so the
            # transfers overlap
            xt0 = xin.tile([128, WPAD], fin)
            nc.sync.dma_start(out=xt0[:, 0:520], in_=xsf[0:128, 0:520])
            nc.sync.dma_start(out=xt0[:, 520:WPAD], in_=xsf[0:128, 520:WPAD])
            wt = wp.tile([128, 3 * 128], fin)
            nc.scalar.dma_start(out=wt, in_=wpk)

            # PE p-state ramps to 2.4 GHz only after ~3us of continuous
            # execution: burn the preamble (waiting on xt0) with dummy
            # matmuls on a zeroed tile so real matmuls start at full clock
            dz = wp.tile([128, 512], fin, name="dz")
            nc.gpsimd.memset(dz, 0.0)
            ptw = ps.tile([128, 512], f32, name="pt0")
            for i in range(6):
                nc.tensor.matmul(ptw, dz[:, 0:128], dz, start=True, stop=True)

            for u in range(groups):
                if u == 0:
                    xt = xt0
                else:
                    xt = xin.tile([128, WPAD], fin)
                    nc.sync.dma_start(
                        out=xt, in_=xsf[adv * C * u : adv * C * u + 128]
                    )
                pt = [ps.tile([128, 512], f32, name=f"pt{h}") for h in range(2)]
                if u == 0:
                    # h-major so the first 3 matmuls only need xt0's first half
                    for h in range(2):
                        for kw in range(3):
                            nc.tensor.matmul(
                                pt[h],
                                wt[:, kw * 128 : (kw + 1) * 128],
                                xt[:, h * 512 + kw : h * 512 + kw + 512],
                                start=(kw == 0),
                                stop=(kw == 2),
                            )
                elif order == "kw":
                    for kw in range(3):
                        for h in range(2):
                            nc.tensor.matmul(
                                pt[h],
                                wt[:, kw * 128 : (kw + 1) * 128],
                                xt[:, h * 512 + kw : h * 512 + kw + 512],
                                start=(kw == 0),
                                stop=(kw == 2),
                            )
                else:
                    for h in range(2):
                        for kw in range(3):
                            nc.tensor.matmul(
                                pt[h],
                                wt[:, kw * 128 : (kw + 1) * 128],
                                xt[:, h * 512 + kw : h * 512 + kw + 512],
                                start=(kw == 0),
                                stop=(kw == 2),
                            )

                stg = stgp.tile([128, W], fout)
                nc.scalar.copy(stg[:, 0:512], pt[0])
                nc.vector.tensor_copy(stg[:, 512:1024], pt[1])
                if adv == 7:
                    eng = nc.gpsimd if u % 2 == 0 else nc.scalar
                    eng.dma_start(out=of[128 * u : 128 * u + 128], in_=stg)
                else:
                    nr = min(6, RPC - 6 * u)
                    eng = nc.gpsimd if u % 2 == 0 else nc.scalar
                    eng.dma_start(
                        out=of[6 * C * u : 6 * C * u + nr * C], in_=stg[0 : nr * C]
                    )
    nc.compile()
    _CACHE[key] = nc
    return nc


def _pack_weights(weight: np.ndarray, adv: int = 7) -> np.ndarray:
    """wpk[(p*16+c), kw*128 + (g*16+o)]:
    g<6:  = W[o,c,kh=p-g,kw]  (full rows, 0<=p-g<=2)
    g==6: = W[o,c,kh=p-6,kw] for p in {6,7}   (own split row, taps kh=0,1)
    g==7: = W[o,c,2,kw] at p==1               (prev window's split row, kh=2)
    """
    wpk = np.zeros((8, C, 3, 8, C), dtype=np.float32)  # [p, c, kw, g, o]
    wt = weight.astype(np.float32).transpose(1, 3, 0, 2)  # [c, kw, o, kh]
    for g in range(6):
        for kh in range(3):
            wpk[g + kh, :, :, g, :] = wt[:, :, :, kh]
    if adv == 7:
        for kh in range(2):
            wpk[6 + kh, :, :, 6, :] = wt[:, :, :, kh]
        wpk[1, :, :, 7, :] = wt[:, :, :, 2]
    return np.ascontiguousarray(wpk.reshape(128, 3 * 128))


def _slice_inputs(x: np.ndarray) -> list[np.ndarray]:
    """Per-core input slices [SROWS, C, WPAD], row-major, zero-padded."""
    xr = x[0].transpose(1, 0, 2)  # [H, C, W]
    gpad = np.zeros((NCORES * RPC + SROWS, C, WPAD), dtype=np.float32)
    gpad[1 : H + 1, :, 1 : W + 1] = xr
    return [np.ascontiguousarray(gpad[RPC * i : RPC * i + SROWS]) for i in range(NCORES)]


def kernel(x: np.ndarray, weight: np.ndarray, _run_kw: dict | None = None,
           _cfg: dict | None = None):
    cfg = {"out_dt": "float16", "order": "kw", "adv": 7, "bufs": (8, 4, 6)}
    cfg.update(_cfg or {})
    adv = cfg["adv"]
    nc = _build_nc(**cfg)
    wpk = _pack_weights(weight, adv=adv).astype(np.float16)
    slices = [s.astype(np.float16) for s in _slice_inputs(np.asarray(x, dtype=np.float32))]
    in_maps = [{"xs": s, "wpk": wpk} for s in slices]
    res = run_bass_kernel_spmd(
        nc, in_maps, core_ids=list(range(NCORES)), **(_run_kw or {})
    )
    if adv == 7:
        full = np.empty((NCORES, RPC, C, W), dtype=np.float32)
        for i in range(NCORES):
            dev = np.asarray(res.results[i]["out"], dtype=np.float32)
            dev = dev.reshape(19, 8, C, W)  # [window, row-block g, ch, w]
            for u in range(19):
                nmain = 6 if u < 18 else 2
                full[i, 7 * u : 7 * u + nmain] = dev[u, :nmain]
            for u in range(18):  # split rows: own partial (u) + kh=2 fixup (u+1)
                full[i, 7 * u + 6] = dev[u, 6] + dev[u + 1, 7]
        full = full.reshape(H, C, W)
    else:
        outs = np.stack([np.asarray(res.results[i]["out"], dtype=np.float32)
                         for i in range(NCORES)])  # [i, r*C, w]
        full = outs.reshape(NCORES, RPC, C, W).reshape(H, C, W)
    full = full.transpose(1, 0, 2).reshape(C, H, W)
    if _run_kw:
        kernel.last_results = res
    return full
